# revision 1
# baseline (speedup 1.0000x reference)
"""Trainium2 Bass kernel for LlamaAttention (B=2, S=2048, D=2048, H=16, HD=128).

Sharding: tensor-parallel over heads. Each of the 8 cores computes 2 heads:
q/k/v projections for its 256-column slice of Wq/Wk/Wv, rope, causal-masked
softmax attention, AV, and a partial output projection with its 256-row slice
of Wo. The 8 partial [B*S, D] outputs are summed on the host.

All matmul/DMA traffic is bf16 (PSUM accumulation fp32): fp32r matmuls run
~2.2 cycles/row on TRN2 hardware while bf16 runs 1.0, and bf16 halves HBM
traffic. q/k are computed feature-major (contraction dim on partitions) with
rope folded into the PSUM eviction; v is computed token-major directly
(lhsT = hs tile, rhs = Wv slice) so no PE transpose or DRAM round-trip is
needed. Softmax has no max subtraction: P = exp(scale*S) * em where em is a
{0,1} (shifted-exp for general masks) factor; for exact-causal masks the 16
diagonal tiles per batch use only 4 distinct [128,512] patterns kept in SBUF.
Row sums via a PE ones-matmul; normalization is folded into the PSUM
eviction of the attention output. Output-projection blocks are interleaved
into the attention loop at lag-1 so the PE stays dense (HAM stays at 8/8).
"""

import os
import sys
from contextlib import ExitStack

import numpy as np

for _p in ("/opt/trn_rl_repo",):
    if _p not in sys.path:
        sys.path.insert(0, _p)

import ml_dtypes  # noqa: E402

import concourse.bass as bass  # noqa: E402
import concourse.tile as tile  # noqa: E402
from concourse import bacc, mybir  # noqa: E402

B, S, D, H, HD = 2, 2048, 2048, 16, 128
T = B * S                    # 4096 tokens total
NCORES = 8
HPC = H // NCORES            # 2 heads per core
JC = HPC * HD                # 256 per-core feature columns
P = 128
TB = 512                     # token block for projections
NTB = T // TB                # 8
KT = D // P                  # 16 contraction tiles of 128
TQB = 512                    # tq block in attention
NTQB = S // TQB              # 4 per batch
NTK = S // P                 # 16 tk tiles per batch
TTB = TB // P                # 4 token tiles of 128 per projection block
SCALE = 1.0 / float(np.sqrt(HD))
ROPE_THETA = 10000.0

F32 = mybir.dt.float32
BF16 = mybir.dt.bfloat16

# tile classes
CLS_SKIP, CLS_ZERO, CLS_MIXED = 0, 1, 2

_prog_cache: dict[bytes, tuple] = {}


def _build_program(cls: np.ndarray, causal: bool):
    """Build the SPMD Bass program. cls: [B, NTK, NTQB] int8 tile classes
    (identical for every core — the mask does not depend on the head).
    causal: the mixed tiles match the 4 canonical diagonal patterns."""
    has_mixed = bool((cls == CLS_MIXED).any())

    nc = bacc.Bacc(
        "TRN2",
        target_bir_lowering=False,
        debug=False,
        enable_asserts=True,
        num_devices=NCORES,
    )

    hs_d = nc.dram_tensor("hs", [NTB, P, KT * TB], BF16, kind="ExternalInput").ap()
    wq_d = nc.dram_tensor("wq", [P, KT * JC], BF16, kind="ExternalInput").ap()
    wk_d = nc.dram_tensor("wk", [P, KT * JC], BF16, kind="ExternalInput").ap()
    wv_d = nc.dram_tensor("wv", [P, KT * JC], BF16, kind="ExternalInput").ap()
    wo_d = nc.dram_tensor("wo", [P, HPC * D], BF16, kind="ExternalInput").ap()
    cos_d = nc.dram_tensor("cosT", [HD, T], BF16, kind="ExternalInput").ap()
    sin_d = nc.dram_tensor("sinT", [HD, T], BF16, kind="ExternalInput").ap()
    em_d = None
    if has_mixed:
        if causal:
            em_d = nc.dram_tensor("emT", [P, 4 * TQB], BF16, kind="ExternalInput").ap()
        else:
            em_d = nc.dram_tensor("emT", [B, S, S], BF16, kind="ExternalInput").ap()
    out_d = nc.dram_tensor("out", [T, D], BF16, kind="ExternalOutput").ap()

    with tile.TileContext(nc) as tc, ExitStack() as ctx:
        # ---------- long-lived tiles ----------
        persist = ctx.enter_context(tc.tile_pool(name="persist", bufs=1))
        qT = persist.tile([P, HPC * T], BF16)     # (j2, t) feature-major q
        kT = persist.tile([P, HPC * T], BF16)
        aT = persist.tile([P, HPC * T], BF16)     # attn out^T (j2, t)
        v_sb = persist.tile([P, B * NTK * JC], BF16)  # token-major v
        ones_t = persist.tile([P, 32], BF16)
        nc.any.memset(ones_t[:], 1.0)

        qT_v = qT[:].rearrange("p (j2 t) -> p j2 t", j2=HPC)
        kT_v = kT[:].rearrange("p (j2 t) -> p j2 t", j2=HPC)
        aT_v = aT[:].rearrange("p (j2 t) -> p j2 t", j2=HPC)

        wpool = ctx.enter_context(tc.tile_pool(name="wpool", bufs=1))
        wq_s = wpool.tile([P, KT * JC], BF16)
        wk_s = wpool.tile([P, KT * JC], BF16)
        wv_s = wpool.tile([P, KT * JC], BF16)
        wo_s = wpool.tile([P, HPC * D], BF16)
        cos_s = wpool.tile([HD, T], BF16)
        sin_s = wpool.tile([HD, T], BF16)
        em4_s = None
        if has_mixed and causal:
            em4_s = wpool.tile([P, 4 * TQB], BF16)
        # wq first so the first matmul group can start ASAP; the rest of the
        # preamble DMAs are traced after hs[0]'s inside the phase-1 block.
        nc.sync.dma_start(wq_s[:], wq_d)

        wq_v, wk_v, wv_v = [
            w[:].rearrange("p (kt j) -> p kt j", j=JC)
            for w in (wq_s, wk_s, wv_s)
        ]
        wo_sv = wo_s[:].rearrange("p (j2 n) -> p j2 n", j2=HPC)

        # ---------- shared work pools (both phases interleave) --------------
        hstp = ctx.enter_context(tc.tile_pool(name="hstp", bufs=2))
        rstg = ctx.enter_context(tc.tile_pool(name="rstg", bufs=3))
        ptp = ctx.enter_context(tc.tile_pool(name="ptp", bufs=6))
        emp = ctx.enter_context(tc.tile_pool(name="emp", bufs=2))
        ssbp = ctx.enter_context(tc.tile_pool(name="ssb", bufs=2))
        rcp = ctx.enter_context(tc.tile_pool(name="rcp", bufs=2))
        rbp = ctx.enter_context(tc.tile_pool(name="rbp", bufs=2))
        ogp = ctx.enter_context(tc.tile_pool(name="ogp", bufs=2))
        # PSUM: 8 banks total. "big" = q/k projection groups + attention
        # score tiles; "vop" = v projection groups + output projection.
        bigp = ctx.enter_context(tc.tile_pool(name="bigp", bufs=2,
                                              space="PSUM"))
        vopp = ctx.enter_context(tc.tile_pool(name="vopp", bufs=2,
                                              space="PSUM"))
        ops = ctx.enter_context(tc.tile_pool(name="opsum", bufs=1,
                                             space="PSUM"))
        sups = ctx.enter_context(tc.tile_pool(name="supsum", bufs=1,
                                              space="PSUM"))

        pend = {}

        def proj_block(tb, hst0=None):
            # q/k (feature-major + rope) and v (token-major) for 512 tokens
            if hst0 is not None:
                hst = hst0
            else:
                hst = hstp.tile([P, KT * TB], BF16, tag="hst")
                nc.sync.dma_start(hst[:], hs_d[tb])
            hv = hst[:].rearrange("p (kt t) -> p kt t", t=TB)
            tsl = slice(tb * TB, (tb + 1) * TB)
            for pi, w_v in ((0, wq_v), (1, wk_v)):
                for j2 in range(HPC):
                    ps = bigp.tile([P, TB], F32, tag="big")
                    for kt in range(KT):
                        nc.tensor.matmul(
                            ps[:],
                            lhsT=w_v[:, kt, j2 * P:(j2 + 1) * P],
                            rhs=hv[:, kt, :],
                            start=(kt == 0),
                            stop=(kt == KT - 1),
                        )
                    # rope folded into the PSUM eviction; sin_s is
                    # pre-signed (lower half negative) so one add works:
                    # out = x*cos + cross(x)*sin_signed
                    dst = (qT_v if pi == 0 else kT_v)[:, j2, tsl]
                    t1 = rstg.tile([P, TB], BF16, tag="ropeA")
                    t2 = rstg.tile([P, TB], BF16, tag="ropeB")
                    nc.vector.tensor_mul(t1[:], ps[:], cos_s[:, tsl])
                    nc.vector.tensor_mul(
                        t2[0:64, :], ps[64:128, :], sin_s[0:64, tsl])
                    nc.vector.tensor_mul(
                        t2[64:128, :], ps[0:64, :], sin_s[64:128, tsl])
                    nc.vector.tensor_add(dst, t1[:], t2[:])
                    pump()
            # v token-major: lhsT = hs tile, rhs = Wv slice
            for tt in range(TTB):
                vp = vopp.tile([P, JC], F32, tag="vop", name="vp")
                for kt in range(KT):
                    nc.tensor.matmul(
                        vp[:],
                        lhsT=hv[:, kt, tt * P:(tt + 1) * P],
                        rhs=wv_v[:, kt, :],
                        start=(kt == 0),
                        stop=(kt == KT - 1),
                    )
                g = tb * TTB + tt              # global token tile = b*NTK+tk
                nc.scalar.copy(v_sb[:, g * JC:(g + 1) * JC], vp[:])
                pump()

        def attn_block(b, tqb):
            tq0 = b * S + tqb * TQB
            qsl = slice(tq0, tq0 + TQB)
            live = [tk for tk in range(NTK) if cls[b, tk, tqb] != CLS_SKIP]
            o_ps = [ops.tile([P, TQB], F32, tag=f"o{h}", name=f"o_ps{h}")
                    for h in range(HPC)]
            s_ps = [sups.tile([1, TQB], F32, tag=f"s{h}", name=f"s_ps{h}")
                    for h in range(HPC)]
            for i, tk in enumerate(live):
                mixed = cls[b, tk, tqb] == CLS_MIXED
                em_ap = None
                c0 = 0
                if mixed:
                    if causal:
                        r = tk - 4 * tqb
                        # the first 128*r query columns of a diagonal tile
                        # are fully masked: skip them in every engine
                        c0 = P * r
                        em_ap = em4_s[:, r * TQB + c0:(r + 1) * TQB]
                    else:
                        em = emp.tile([P, TQB], BF16, tag="em")
                        nc.sync.dma_start(
                            em[:],
                            em_d[b, tk * P:(tk + 1) * P,
                                 tqb * TQB:(tqb + 1) * TQB],
                        )
                        em_ap = em[:]
                W = TQB - c0
                for h in range(HPC):
                    st = bigp.tile([P, TQB], F32, tag="big", name="st")
                    nc.tensor.matmul(
                        st[:, 0:W],
                        lhsT=kT_v[:, h, b * S + tk * P:
                                  b * S + (tk + 1) * P],
                        rhs=qT_v[:, h, tq0 + c0:tq0 + TQB],
                        start=True, stop=True,
                    )
                    pt = ptp.tile([P, TQB], BF16, tag="pt")
                    nc.scalar.activation(
                        pt[:, c0:TQB], st[:, 0:W],
                        mybir.ActivationFunctionType.Exp,
                        scale=SCALE,
                    )
                    if em_ap is not None:
                        nc.vector.tensor_mul(pt[:, c0:TQB],
                                             pt[:, c0:TQB], em_ap)
                    nc.tensor.matmul(
                        o_ps[h][:, c0:TQB],
                        lhsT=v_sb[:, (b * NTK + tk) * JC + h * P:
                                  (b * NTK + tk) * JC + (h + 1) * P],
                        rhs=pt[:, c0:TQB],
                        start=(i == 0), stop=(i == len(live) - 1),
                    )
                    nc.tensor.matmul(
                        s_ps[h][0:1, c0:TQB],
                        lhsT=ones_t[:, 0:1],
                        rhs=pt[:, c0:TQB],
                        start=(i == 0), stop=(i == len(live) - 1),
                    )
                pump()
            # evict unnormalized to SBUF immediately (frees PSUM banks);
            # the reciprocal/broadcast/normalize chain is traced one block
            # later (chain_block) so no engine FIFO idles on it.
            ssb = []
            for h in range(HPC):
                nc.scalar.copy(aT_v[:, h, qsl], o_ps[h][:])
                s_sb = ssbp.tile([1, TQB], F32, tag=f"ssb{h}")
                nc.scalar.copy(s_sb[:], s_ps[h][:])
                ssb.append(s_sb)
            pend[(b, tqb)] = ssb

        def chain_block(b, tqb):
            tq0 = b * S + tqb * TQB
            qsl = slice(tq0, tq0 + TQB)
            ssb = pend.pop((b, tqb))
            for h in range(HPC):
                rc = rcp.tile([1, TQB], F32, tag=f"rc{h}")
                nc.vector.reciprocal(rc[:], ssb[h][:])
                rb = rbp.tile([P, TQB], F32, tag=f"rb{h}")
                nc.gpsimd.partition_broadcast(rb[:], rc[:])
                nc.vector.tensor_mul(aT_v[:, h, qsl],
                                     aT_v[:, h, qsl], rb[:])

        outq = []

        def out_unit(b, tqb, tt):
            # one token tile of output projection; PSUM evictions alternate
            # ACT/DVE so neither FIFO gates the PE.
            tb32 = (b * S + tqb * TQB) // P + tt
            og = ogp.tile([P, D], BF16, tag="og")
            for nb in range(D // 512):
                op = vopp.tile([P, 512], F32, tag="vop", name="op")
                for h in range(HPC):
                    nc.tensor.matmul(
                        op[:],
                        lhsT=aT_v[:, h, tb32 * P:(tb32 + 1) * P],
                        rhs=wo_sv[:, h, nb * 512:(nb + 1) * 512],
                        start=(h == 0), stop=(h == HPC - 1),
                    )
                osl = og[:, nb * 512:(nb + 1) * 512]
                nc.vector.tensor_copy(osl, op[:])
            nc.sync.dma_start(
                out_d[tb32 * P:(tb32 + 1) * P, :], og[:])

        def out_block(b, tqb):
            # queue the 4 token-tile units; they are traced ("pumped") one at
            # a time between attention tiles / projection groups so bubbles
            # in those pipelines get filled with independent outproj matmuls
            for tt in range(TQB // P):
                outq.append((b, tqb, tt))

        def pump():
            if outq:
                out_unit(*outq.pop(0))

        # ---------- schedule -----------------------------------------------
        # proj(b0) -> attn(b0) (outproj lag-1) -> proj(b1) -> attn(b1).
        # attention(b0)'s softmax/eviction chains drain on ACT/DVE/GPSIMD
        # while projection(b1) keeps the PE dense, and vice versa.
        hst0 = hstp.tile([P, KT * TB], BF16, tag="hst")
        nc.sync.dma_start(hst0[:], hs_d[0])
        nc.sync.dma_start(wk_s[:], wk_d)
        nc.sync.dma_start(wv_s[:], wv_d)
        nc.sync.dma_start(cos_s[:], cos_d)
        nc.sync.dma_start(sin_s[:], sin_d)
        nc.sync.dma_start(wo_s[:], wo_d)
        if em4_s is not None:
            nc.sync.dma_start(em4_s[:], em_d)
        proj_block(0, hst0=hst0)
        for tb in range(1, NTB // 2):
            proj_block(tb)
        blocks0 = [(0, tqb) for tqb in range(NTQB)]
        blocks1 = [(1, tqb) for tqb in range(NTQB)]
        for k, blk in enumerate(blocks0):
            attn_block(*blk)
            if k >= 1:
                chain_block(*blocks0[k - 1])
                out_block(*blocks0[k - 1])
        proj_block(NTB // 2)
        chain_block(*blocks0[-1])
        out_block(*blocks0[-1])
        for tb in range(NTB // 2 + 1, NTB):
            proj_block(tb)
        while outq:
            pump()
        for k, blk in enumerate(blocks1):
            attn_block(*blk)
            if k >= 1:
                chain_block(*blocks1[k - 1])
                out_block(*blocks1[k - 1])
        chain_block(*blocks1[-1])
        out_block(*blocks1[-1])
        while outq:
            pump()

    nc.compile()
    return nc


def _host_prep(hidden_states, attention_mask, position_ids):
    hs = np.asarray(hidden_states, dtype=np.float32).reshape(T, D)
    # hs_d[tb, p, kt*TB + t] = hs[tb*TB + t, kt*P + p]
    hs_t = np.ascontiguousarray(
        hs.reshape(NTB, TB, KT, P).transpose(0, 3, 2, 1)
        .reshape(NTB, P, KT * TB).astype(ml_dtypes.bfloat16))

    # rope tables gathered by position_ids, feature-major, sign baked into sin
    inv_freq = 1.0 / (ROPE_THETA ** (np.arange(0, HD, 2, dtype=np.float32) / HD))
    pos = np.asarray(position_ids).astype(np.int64)
    maxpos = int(pos.max()) + 1
    t_ar = np.arange(maxpos, dtype=np.float32)
    freqs = np.outer(t_ar, inv_freq)                        # [maxpos, 64]
    emb = np.concatenate([freqs, freqs], axis=-1)           # [maxpos, 128]
    cos_tab = np.cos(emb).astype(np.float32)
    sin_tab = np.sin(emb).astype(np.float32)
    sin_tab[:, :HD // 2] *= -1.0                            # pre-signed
    cos_g = cos_tab[pos]                                    # [B, S, HD]
    sin_g = sin_tab[pos]
    cosT = np.concatenate([cos_g[b].T for b in range(B)], axis=1)  # [HD, T]
    sinT = np.concatenate([sin_g[b].T for b in range(B)], axis=1)
    cosT = np.ascontiguousarray(cosT.astype(ml_dtypes.bfloat16))
    sinT = np.ascontiguousarray(sinT.astype(ml_dtypes.bfloat16))

    # shifted-exp mask, transposed per batch, plus tile classification
    m = np.asarray(attention_mask, dtype=np.float32)[:, 0]  # [B, S(tq), S(tk)]
    rowmax = m.max(axis=-1, keepdims=True)
    em = np.exp(m - rowmax)                                 # [B, tq, tk] in [0,1]
    emT = np.ascontiguousarray(em.transpose(0, 2, 1))       # [B, tk, tq]
    emr = emT.reshape(B, NTK, P, NTQB, TQB)
    tmax = emr.max(axis=(2, 4))                             # [B, NTK, NTQB]
    tmin = emr.min(axis=(2, 4))
    cls = np.full((B, NTK, NTQB), CLS_MIXED, dtype=np.int8)
    cls[tmax == 0.0] = CLS_SKIP
    cls[(tmin == 1.0) & (tmax == 1.0)] = CLS_ZERO
    # guard: a fully-skipped tq column would leave PSUM unwritten
    for b in range(B):
        for tqb in range(NTQB):
            if (cls[b, :, tqb] == CLS_SKIP).all():
                cls[b, 0, tqb] = CLS_MIXED

    # causal fast path: every mixed tile sits on the diagonal (tk = 4*tqb+r,
    # r in 0..3) and matches the canonical pattern 1[tql >= tkl + 128*r]
    tkl = np.arange(P)[:, None]
    tql = np.arange(TQB)[None, :]
    pat = np.stack([(tql >= tkl + P * r).astype(np.float32)
                    for r in range(4)])                     # [4, P, TQB]
    causal = True
    for b in range(B):
        for tk in range(NTK):
            for tqb in range(NTQB):
                if cls[b, tk, tqb] != CLS_MIXED:
                    continue
                r = tk - 4 * tqb
                tile_ = emr[b, tk, :, tqb, :]
                if not (0 <= r < 4) or not np.array_equal(tile_, pat[r]):
                    causal = False
    em_out = (np.ascontiguousarray(
                  pat.transpose(1, 0, 2).reshape(P, 4 * TQB)
                  .astype(ml_dtypes.bfloat16)) if causal
              else np.ascontiguousarray(emT.astype(ml_dtypes.bfloat16)))
    return hs_t, cosT, sinT, em_out, cls, causal


def kernel(hidden_states, attention_mask, position_ids, Wq, Wk, Wv, Wo):
    hs_t, cosT, sinT, em_out, cls, causal = _host_prep(
        hidden_states, attention_mask, position_ids)

    key = cls.tobytes() + bytes([causal])
    if key not in _prog_cache:
        _prog_cache[key] = _build_program(cls, causal)
    nc = _prog_cache[key]
    has_mixed = bool((cls == CLS_MIXED).any())

    Wq = np.asarray(Wq, dtype=np.float32)
    Wk = np.asarray(Wk, dtype=np.float32)
    Wv = np.asarray(Wv, dtype=np.float32)
    Wo = np.asarray(Wo, dtype=np.float32)

    in_maps = []
    for c in range(NCORES):
        jsl = slice(c * JC, (c + 1) * JC)

        def sb_w(w):  # [D, JC] -> SBUF layout [128, KT*JC]
            return np.ascontiguousarray(
                w.reshape(KT, P, JC).transpose(1, 0, 2).reshape(P, KT * JC)
                .astype(ml_dtypes.bfloat16))

        m = {
            "hs": hs_t,
            "wq": sb_w(Wq[:, jsl]),
            "wk": sb_w(Wk[:, jsl]),
            "wv": sb_w(Wv[:, jsl]),
            "wo": np.ascontiguousarray(
                Wo[jsl, :].reshape(HPC, P, D).transpose(1, 0, 2)
                .reshape(P, HPC * D).astype(ml_dtypes.bfloat16)),
            "cosT": cosT,
            "sinT": sinT,
        }
        if has_mixed:
            m["emT"] = em_out
        in_maps.append(m)

    if os.environ.get("KERNEL_SIM"):
        from concourse.bass_interp import CoreSim
        outs = []
        for c in range(int(os.environ.get("KERNEL_SIM_CORES", "1"))):
            sim = CoreSim(nc, require_finite=False, require_nnan=True)
            for k, v in in_maps[c].items():
                sim.tensor(k)[:] = v
            sim.simulate(check_with_hw=False)
            outs.append(np.array(sim.tensor("out")).astype(np.float32))
        total = np.sum(np.stack(outs, 0), axis=0)
        return total.reshape(B, S, D).astype(np.float32)

    from concourse.bass_utils import run_bass_kernel_spmd
    trace = bool(os.environ.get("KERNEL_TRACE"))
    res = run_bass_kernel_spmd(
        nc, in_maps, core_ids=list(range(NCORES)), trace=trace)
    if trace and res.exec_time_ns is not None:
        print(f"HW exec time: {res.exec_time_ns} ns")
        kernel.last_exec_time_ns = res.exec_time_ns
        kernel.last_trace = res.instructions_and_trace
    partials = np.stack([r["out"].astype(np.float32) for r in res.results],
                        axis=0)
    total = partials.sum(axis=0)
    return total.reshape(B, S, D).astype(np.float32)



# revision 4
# speedup vs baseline: 1.3452x; 1.3452x over previous
"""Trainium2 Bass kernel for LlamaAttention (B=2, S=2048, D=2048, H=16, HD=128).

Sharding: tensor-parallel over heads. Each of the 8 cores computes 2 heads:
q/k/v projections for its 256-column slice of Wq/Wk/Wv, rope, causal-masked
softmax attention, AV, and a partial output projection with its 256-row slice
of Wo. The 8 partial [B*S, D] outputs are summed on the host.

All matmul/DMA traffic is bf16 (PSUM accumulation fp32): fp32r matmuls run
~2.2 cycles/row on TRN2 hardware while bf16 runs 1.0, and bf16 halves HBM
traffic. q/k are computed feature-major (contraction dim on partitions) with
rope folded into the PSUM eviction; v is computed token-major directly
(lhsT = hs tile, rhs = Wv slice) so no PE transpose or DRAM round-trip is
needed. Softmax has no max subtraction: P = exp(scale*S) * em where em is a
{0,1} (shifted-exp for general masks) factor; for exact-causal masks the 16
diagonal tiles per batch use only 4 distinct [128,512] patterns kept in SBUF.
Row sums via a PE ones-matmul; normalization is folded into the PSUM
eviction of the attention output. Output-projection blocks are interleaved
into the attention loop at lag-1 so the PE stays dense (HAM stays at 8/8).
"""

import os
import sys
from contextlib import ExitStack

import numpy as np

for _p in ("/opt/trn_rl_repo",):
    if _p not in sys.path:
        sys.path.insert(0, _p)

import ml_dtypes  # noqa: E402

import concourse.bass as bass  # noqa: E402
import concourse.tile as tile  # noqa: E402
from concourse import bacc, mybir  # noqa: E402

B, S, D, H, HD = 2, 2048, 2048, 16, 128
T = B * S                    # 4096 tokens total
NCORES = 8
HPC = H // NCORES            # 2 heads per core
JC = HPC * HD                # 256 per-core feature columns
P = 128
TB = 512                     # token block for projections
NTB = T // TB                # 8
KT = D // P                  # 16 contraction tiles of 128
TQB = 512                    # tq block in attention
NTQB = S // TQB              # 4 per batch
NTK = S // P                 # 16 tk tiles per batch
TTB = TB // P                # 4 token tiles of 128 per projection block
SCALE = 1.0 / float(np.sqrt(HD))
ROPE_THETA = 10000.0

F32 = mybir.dt.float32
BF16 = mybir.dt.bfloat16

# tile classes
CLS_SKIP, CLS_ZERO, CLS_MIXED = 0, 1, 2

_prog_cache: dict[bytes, tuple] = {}


def _build_program(cls: np.ndarray, causal: bool):
    """Build the SPMD Bass program. cls: [B, NTK, NTQB] int8 tile classes
    (identical for every core — the mask does not depend on the head).
    causal: the mixed tiles match the 4 canonical diagonal patterns."""
    has_mixed = bool((cls == CLS_MIXED).any())

    nc = bacc.Bacc(
        "TRN2",
        target_bir_lowering=False,
        debug=False,
        enable_asserts=True,
        num_devices=NCORES,
    )

    hs_d = nc.dram_tensor("hs", [NTB, P, KT * TB], BF16, kind="ExternalInput").ap()
    wq_d = nc.dram_tensor("wq", [P, KT * JC], BF16, kind="ExternalInput").ap()
    wk_d = nc.dram_tensor("wk", [P, KT * JC], BF16, kind="ExternalInput").ap()
    wv_d = nc.dram_tensor("wv", [P, KT * JC], BF16, kind="ExternalInput").ap()
    wo_d = nc.dram_tensor("wo", [P, HPC * D], BF16, kind="ExternalInput").ap()
    cos_d = nc.dram_tensor("cosT", [HD, T], BF16, kind="ExternalInput").ap()
    sin_d = nc.dram_tensor("sinT", [HD, T], BF16, kind="ExternalInput").ap()
    em_d = None
    if has_mixed:
        if causal:
            em_d = nc.dram_tensor("emT", [P, 4 * TQB], BF16, kind="ExternalInput").ap()
        else:
            em_d = nc.dram_tensor("emT", [B, S, S], BF16, kind="ExternalInput").ap()
    out_d = nc.dram_tensor("out", [T, D], BF16, kind="ExternalOutput").ap()

    with tile.TileContext(nc) as tc, ExitStack() as ctx:
        # ---------- long-lived tiles ----------
        persist = ctx.enter_context(tc.tile_pool(name="persist", bufs=1))
        qT = persist.tile([P, HPC * T], BF16)     # (j2, t) feature-major q
        kT = persist.tile([P, HPC * T], BF16)
        aT = persist.tile([P, HPC * T], BF16)     # attn out^T (j2, t)
        v_sb = persist.tile([P, B * NTK * JC], BF16)  # token-major v
        ones_t = persist.tile([P, 32], BF16)
        nc.any.memset(ones_t[:], 1.0)

        qT_v = qT[:].rearrange("p (j2 t) -> p j2 t", j2=HPC)
        kT_v = kT[:].rearrange("p (j2 t) -> p j2 t", j2=HPC)
        aT_v = aT[:].rearrange("p (j2 t) -> p j2 t", j2=HPC)

        wpool = ctx.enter_context(tc.tile_pool(name="wpool", bufs=1))
        wq_s = wpool.tile([P, KT * JC], BF16)
        wk_s = wpool.tile([P, KT * JC], BF16)
        wv_s = wpool.tile([P, KT * JC], BF16)
        wo_s = wpool.tile([P, HPC * D], BF16)
        cos_s = wpool.tile([HD, T], BF16)
        sin_s = wpool.tile([HD, T], BF16)
        em4_s = None
        if has_mixed and causal:
            em4_s = wpool.tile([P, 4 * TQB], BF16)
        # wq first so the first matmul group can start ASAP; the rest of the
        # preamble DMAs are traced after hs[0]'s inside the phase-1 block.
        nc.sync.dma_start(wq_s[:], wq_d)

        wq_v, wk_v, wv_v = [
            w[:].rearrange("p (kt j) -> p kt j", j=JC)
            for w in (wq_s, wk_s, wv_s)
        ]
        wo_sv = wo_s[:].rearrange("p (j2 n) -> p j2 n", j2=HPC)

        # ---------- shared work pools (both phases interleave) --------------
        hstp = ctx.enter_context(tc.tile_pool(name="hstp", bufs=2))
        rstg = ctx.enter_context(tc.tile_pool(name="rstg", bufs=3))
        ptp = ctx.enter_context(tc.tile_pool(name="ptp", bufs=6))
        emp = ctx.enter_context(tc.tile_pool(name="emp", bufs=2))
        ssbp = ctx.enter_context(tc.tile_pool(name="ssb", bufs=2))
        rcp = ctx.enter_context(tc.tile_pool(name="rcp", bufs=2))
        rbp = ctx.enter_context(tc.tile_pool(name="rbp", bufs=2))
        ogp = ctx.enter_context(tc.tile_pool(name="ogp", bufs=2))
        # PSUM: 8 banks total. "big" = q/k projection groups + attention
        # score tiles; "vop" = v projection groups + output projection.
        bigp = ctx.enter_context(tc.tile_pool(name="bigp", bufs=2,
                                              space="PSUM"))
        vopp = ctx.enter_context(tc.tile_pool(name="vopp", bufs=2,
                                              space="PSUM"))
        ops = ctx.enter_context(tc.tile_pool(name="opsum", bufs=1,
                                             space="PSUM"))
        sups = ctx.enter_context(tc.tile_pool(name="supsum", bufs=1,
                                              space="PSUM"))

        pend = {}

        def proj_block(tb, hst0=None):
            # q/k (feature-major + rope) and v (token-major) for 512 tokens
            if hst0 is not None:
                hst = hst0
            else:
                hst = hstp.tile([P, KT * TB], BF16, tag="hst")
                nc.sync.dma_start(hst[:], hs_d[tb])
            hv = hst[:].rearrange("p (kt t) -> p kt t", t=TB)
            tsl = slice(tb * TB, (tb + 1) * TB)
            for pi, w_v in ((0, wq_v), (1, wk_v)):
                for j2 in range(HPC):
                    ps = bigp.tile([P, TB], F32, tag="big")
                    for kt in range(KT):
                        nc.tensor.matmul(
                            ps[:],
                            lhsT=w_v[:, kt, j2 * P:(j2 + 1) * P],
                            rhs=hv[:, kt, :],
                            start=(kt == 0),
                            stop=(kt == KT - 1),
                        )
                    # rope folded into the PSUM eviction; sin_s is
                    # pre-signed (lower half negative) so one add works:
                    # out = x*cos + cross(x)*sin_signed
                    dst = (qT_v if pi == 0 else kT_v)[:, j2, tsl]
                    t1 = rstg.tile([P, TB], BF16, tag="ropeA")
                    t2 = rstg.tile([P, TB], BF16, tag="ropeB")
                    nc.vector.tensor_mul(t1[:], ps[:], cos_s[:, tsl])
                    nc.vector.tensor_mul(
                        t2[0:64, :], ps[64:128, :], sin_s[0:64, tsl])
                    nc.vector.tensor_mul(
                        t2[64:128, :], ps[0:64, :], sin_s[64:128, tsl])
                    nc.vector.tensor_add(dst, t1[:], t2[:])
                    pump()
            # v token-major: lhsT = hs tile, rhs = Wv slice
            for tt in range(TTB):
                vp = vopp.tile([P, JC], F32, tag="vop", name="vp")
                for kt in range(KT):
                    nc.tensor.matmul(
                        vp[:],
                        lhsT=hv[:, kt, tt * P:(tt + 1) * P],
                        rhs=wv_v[:, kt, :],
                        start=(kt == 0),
                        stop=(kt == KT - 1),
                    )
                g = tb * TTB + tt              # global token tile = b*NTK+tk
                nc.scalar.copy(v_sb[:, g * JC:(g + 1) * JC], vp[:])
                pump()

        def attn_block(b, tqb):
            tq0 = b * S + tqb * TQB
            qsl = slice(tq0, tq0 + TQB)
            live = [tk for tk in range(NTK) if cls[b, tk, tqb] != CLS_SKIP]
            o_ps = [ops.tile([P, TQB], F32, tag=f"o{h}", name=f"o_ps{h}")
                    for h in range(HPC)]
            s_ps = [sups.tile([1, TQB], F32, tag=f"s{h}", name=f"s_ps{h}")
                    for h in range(HPC)]
            for i, tk in enumerate(live):
                mixed = cls[b, tk, tqb] == CLS_MIXED
                em_ap = None
                c0 = 0
                if mixed:
                    if causal:
                        r = tk - 4 * tqb
                        # the first 128*r query columns of a diagonal tile
                        # are fully masked: skip them in every engine
                        c0 = P * r
                        em_ap = em4_s[:, r * TQB + c0:(r + 1) * TQB]
                    else:
                        em = emp.tile([P, TQB], BF16, tag="em")
                        nc.sync.dma_start(
                            em[:],
                            em_d[b, tk * P:(tk + 1) * P,
                                 tqb * TQB:(tqb + 1) * TQB],
                        )
                        em_ap = em[:]
                W = TQB - c0
                pts = []
                for h in range(HPC):
                    st = bigp.tile([P, TQB], F32, tag="big", name="st")
                    nc.tensor.matmul(
                        st[:, 0:W],
                        lhsT=kT_v[:, h, b * S + tk * P:
                                  b * S + (tk + 1) * P],
                        rhs=qT_v[:, h, tq0 + c0:tq0 + TQB],
                        start=True, stop=True,
                    )
                    pt = ptp.tile([P, TQB], BF16, tag="pt")
                    nc.scalar.activation(
                        pt[:, c0:TQB], st[:, 0:W],
                        mybir.ActivationFunctionType.Exp,
                        scale=SCALE,
                    )
                    if em_ap is not None:
                        nc.vector.tensor_mul(pt[:, c0:TQB],
                                             pt[:, c0:TQB], em_ap)
                    pts.append(pt)
                for h in range(HPC):
                    nc.tensor.matmul(
                        o_ps[h][:, c0:TQB],
                        lhsT=v_sb[:, (b * NTK + tk) * JC + h * P:
                                  (b * NTK + tk) * JC + (h + 1) * P],
                        rhs=pts[h][:, c0:TQB],
                        start=(i == 0), stop=(i == len(live) - 1),
                    )
                for h in range(HPC):
                    nc.tensor.matmul(
                        s_ps[h][0:1, c0:TQB],
                        lhsT=ones_t[:, 0:1],
                        rhs=pts[h][:, c0:TQB],
                        start=(i == 0), stop=(i == len(live) - 1),
                    )
                pump()
            # evict unnormalized to SBUF immediately (frees PSUM banks);
            # the reciprocal/broadcast/normalize chain is traced one block
            # later (chain_block) so no engine FIFO idles on it.
            ssb = []
            for h in range(HPC):
                nc.scalar.copy(aT_v[:, h, qsl], o_ps[h][:])
                s_sb = ssbp.tile([1, TQB], F32, tag=f"ssb{h}")
                nc.scalar.copy(s_sb[:], s_ps[h][:])
                ssb.append(s_sb)
            pend[(b, tqb)] = ssb

        def chain_block(b, tqb):
            tq0 = b * S + tqb * TQB
            qsl = slice(tq0, tq0 + TQB)
            ssb = pend.pop((b, tqb))
            for h in range(HPC):
                rc = rcp.tile([1, TQB], F32, tag=f"rc{h}")
                nc.vector.reciprocal_approx_fast(rc[:], ssb[h][:])
                rb = rbp.tile([P, TQB], F32, tag=f"rb{h}")
                nc.gpsimd.partition_broadcast(rb[:], rc[:])
                nc.vector.tensor_mul(aT_v[:, h, qsl],
                                     aT_v[:, h, qsl], rb[:])

        outq = []

        def out_unit(b, tqb, tt):
            # one token tile of output projection; PSUM evictions alternate
            # ACT/DVE so neither FIFO gates the PE.
            tb32 = (b * S + tqb * TQB) // P + tt
            og = ogp.tile([P, D], BF16, tag="og")
            for nb in range(D // 512):
                op = vopp.tile([P, 512], F32, tag="vop", name="op")
                for h in range(HPC):
                    nc.tensor.matmul(
                        op[:],
                        lhsT=aT_v[:, h, tb32 * P:(tb32 + 1) * P],
                        rhs=wo_sv[:, h, nb * 512:(nb + 1) * 512],
                        start=(h == 0), stop=(h == HPC - 1),
                    )
                osl = og[:, nb * 512:(nb + 1) * 512]
                if nb % 2 == 0:
                    nc.vector.tensor_copy(osl, op[:])
                else:
                    nc.scalar.copy(osl, op[:])
            nc.sync.dma_start(
                out_d[tb32 * P:(tb32 + 1) * P, :], og[:])

        def out_block(b, tqb):
            # queue the 4 token-tile units; they are traced ("pumped") one at
            # a time between attention tiles / projection groups so bubbles
            # in those pipelines get filled with independent outproj matmuls
            for tt in range(TQB // P):
                outq.append((b, tqb, tt))

        def pump():
            if outq:
                out_unit(*outq.pop(0))

        # ---------- schedule -----------------------------------------------
        # proj(b0) -> attn(b0) (outproj lag-1) -> proj(b1) -> attn(b1).
        # attention(b0)'s softmax/eviction chains drain on ACT/DVE/GPSIMD
        # while projection(b1) keeps the PE dense, and vice versa.
        hst0 = hstp.tile([P, KT * TB], BF16, tag="hst")
        nc.sync.dma_start(hst0[:], hs_d[0])
        nc.sync.dma_start(wk_s[:], wk_d)
        nc.sync.dma_start(wv_s[:], wv_d)
        nc.sync.dma_start(cos_s[:], cos_d)
        nc.sync.dma_start(sin_s[:], sin_d)
        nc.sync.dma_start(wo_s[:], wo_d)
        if em4_s is not None:
            nc.sync.dma_start(em4_s[:], em_d)
        proj_block(0, hst0=hst0)
        for tb in range(1, NTB // 2):
            proj_block(tb)
        blocks0 = [(0, tqb) for tqb in range(NTQB)]
        blocks1 = [(1, tqb) for tqb in range(NTQB)]
        for k, blk in enumerate(blocks0):
            attn_block(*blk)
            if k >= 1:
                chain_block(*blocks0[k - 1])
                out_block(*blocks0[k - 1])
        proj_block(NTB // 2)
        chain_block(*blocks0[-1])
        out_block(*blocks0[-1])
        for tb in range(NTB // 2 + 1, NTB):
            proj_block(tb)
        while outq:
            pump()
        for k, blk in enumerate(blocks1):
            attn_block(*blk)
            if k >= 1:
                chain_block(*blocks1[k - 1])
                out_block(*blocks1[k - 1])
        chain_block(*blocks1[-1])
        out_block(*blocks1[-1])
        while outq:
            pump()

    nc.compile()
    return nc


def _host_prep(hidden_states, attention_mask, position_ids):
    hs = np.asarray(hidden_states, dtype=np.float32).reshape(T, D)
    # hs_d[tb, p, kt*TB + t] = hs[tb*TB + t, kt*P + p]
    hs_t = np.ascontiguousarray(
        hs.reshape(NTB, TB, KT, P).transpose(0, 3, 2, 1)
        .reshape(NTB, P, KT * TB).astype(ml_dtypes.bfloat16))

    # rope tables gathered by position_ids, feature-major, sign baked into sin
    inv_freq = 1.0 / (ROPE_THETA ** (np.arange(0, HD, 2, dtype=np.float32) / HD))
    pos = np.asarray(position_ids).astype(np.int64)
    maxpos = int(pos.max()) + 1
    t_ar = np.arange(maxpos, dtype=np.float32)
    freqs = np.outer(t_ar, inv_freq)                        # [maxpos, 64]
    emb = np.concatenate([freqs, freqs], axis=-1)           # [maxpos, 128]
    cos_tab = np.cos(emb).astype(np.float32)
    sin_tab = np.sin(emb).astype(np.float32)
    sin_tab[:, :HD // 2] *= -1.0                            # pre-signed
    cos_g = cos_tab[pos]                                    # [B, S, HD]
    sin_g = sin_tab[pos]
    cosT = np.concatenate([cos_g[b].T for b in range(B)], axis=1)  # [HD, T]
    sinT = np.concatenate([sin_g[b].T for b in range(B)], axis=1)
    cosT = np.ascontiguousarray(cosT.astype(ml_dtypes.bfloat16))
    sinT = np.ascontiguousarray(sinT.astype(ml_dtypes.bfloat16))

    # shifted-exp mask, transposed per batch, plus tile classification
    m = np.asarray(attention_mask, dtype=np.float32)[:, 0]  # [B, S(tq), S(tk)]
    rowmax = m.max(axis=-1, keepdims=True)
    em = np.exp(m - rowmax)                                 # [B, tq, tk] in [0,1]
    emT = np.ascontiguousarray(em.transpose(0, 2, 1))       # [B, tk, tq]
    emr = emT.reshape(B, NTK, P, NTQB, TQB)
    tmax = emr.max(axis=(2, 4))                             # [B, NTK, NTQB]
    tmin = emr.min(axis=(2, 4))
    cls = np.full((B, NTK, NTQB), CLS_MIXED, dtype=np.int8)
    cls[tmax == 0.0] = CLS_SKIP
    cls[(tmin == 1.0) & (tmax == 1.0)] = CLS_ZERO
    # guard: a fully-skipped tq column would leave PSUM unwritten
    for b in range(B):
        for tqb in range(NTQB):
            if (cls[b, :, tqb] == CLS_SKIP).all():
                cls[b, 0, tqb] = CLS_MIXED

    # causal fast path: every mixed tile sits on the diagonal (tk = 4*tqb+r,
    # r in 0..3) and matches the canonical pattern 1[tql >= tkl + 128*r]
    tkl = np.arange(P)[:, None]
    tql = np.arange(TQB)[None, :]
    pat = np.stack([(tql >= tkl + P * r).astype(np.float32)
                    for r in range(4)])                     # [4, P, TQB]
    causal = True
    for b in range(B):
        for tk in range(NTK):
            for tqb in range(NTQB):
                if cls[b, tk, tqb] != CLS_MIXED:
                    continue
                r = tk - 4 * tqb
                tile_ = emr[b, tk, :, tqb, :]
                if not (0 <= r < 4) or not np.array_equal(tile_, pat[r]):
                    causal = False
    em_out = (np.ascontiguousarray(
                  pat.transpose(1, 0, 2).reshape(P, 4 * TQB)
                  .astype(ml_dtypes.bfloat16)) if causal
              else np.ascontiguousarray(emT.astype(ml_dtypes.bfloat16)))
    return hs_t, cosT, sinT, em_out, cls, causal


def kernel(hidden_states, attention_mask, position_ids, Wq, Wk, Wv, Wo):
    hs_t, cosT, sinT, em_out, cls, causal = _host_prep(
        hidden_states, attention_mask, position_ids)

    key = cls.tobytes() + bytes([causal])
    if key not in _prog_cache:
        _prog_cache[key] = _build_program(cls, causal)
    nc = _prog_cache[key]
    has_mixed = bool((cls == CLS_MIXED).any())

    Wq = np.asarray(Wq, dtype=np.float32)
    Wk = np.asarray(Wk, dtype=np.float32)
    Wv = np.asarray(Wv, dtype=np.float32)
    Wo = np.asarray(Wo, dtype=np.float32)

    in_maps = []
    for c in range(NCORES):
        jsl = slice(c * JC, (c + 1) * JC)

        def sb_w(w):  # [D, JC] -> SBUF layout [128, KT*JC]
            return np.ascontiguousarray(
                w.reshape(KT, P, JC).transpose(1, 0, 2).reshape(P, KT * JC)
                .astype(ml_dtypes.bfloat16))

        m = {
            "hs": hs_t,
            "wq": sb_w(Wq[:, jsl]),
            "wk": sb_w(Wk[:, jsl]),
            "wv": sb_w(Wv[:, jsl]),
            "wo": np.ascontiguousarray(
                Wo[jsl, :].reshape(HPC, P, D).transpose(1, 0, 2)
                .reshape(P, HPC * D).astype(ml_dtypes.bfloat16)),
            "cosT": cosT,
            "sinT": sinT,
        }
        if has_mixed:
            m["emT"] = em_out
        in_maps.append(m)

    if os.environ.get("KERNEL_SIM"):
        from concourse.bass_interp import CoreSim
        outs = []
        for c in range(int(os.environ.get("KERNEL_SIM_CORES", "1"))):
            sim = CoreSim(nc, require_finite=False, require_nnan=True)
            for k, v in in_maps[c].items():
                sim.tensor(k)[:] = v
            sim.simulate(check_with_hw=False)
            outs.append(np.array(sim.tensor("out")).astype(np.float32))
        total = np.sum(np.stack(outs, 0), axis=0)
        return total.reshape(B, S, D).astype(np.float32)

    from concourse.bass_utils import run_bass_kernel_spmd
    trace = bool(os.environ.get("KERNEL_TRACE"))
    res = run_bass_kernel_spmd(
        nc, in_maps, core_ids=list(range(NCORES)), trace=trace)
    if trace and res.exec_time_ns is not None:
        print(f"HW exec time: {res.exec_time_ns} ns")
        kernel.last_exec_time_ns = res.exec_time_ns
        kernel.last_trace = res.instructions_and_trace
    partials = np.stack([r["out"].astype(np.float32) for r in res.results],
                        axis=0)
    total = partials.sum(axis=0)
    return total.reshape(B, S, D).astype(np.float32)



# revision 61
# speedup vs baseline: 1.5265x; 1.1348x over previous
"""Trainium2 Bass kernel for LlamaAttention (B=2, S=2048, D=2048, H=16, HD=128).

Sharding: tensor-parallel over heads. Each of the 8 cores computes 2 heads:
q/k/v projections for its 256-column slice of Wq/Wk/Wv, rope, causal-masked
softmax attention, AV, and a partial output projection with its 256-row slice
of Wo. The 8 partial [B*S, D] outputs are summed on the host.

All matmul/DMA traffic is bf16 (PSUM accumulation fp32): fp32r matmuls run
~2.2 cycles/row on TRN2 hardware while bf16 runs 1.0, and bf16 halves HBM
traffic. q/k are computed feature-major (contraction dim on partitions) with
rope folded into the PSUM eviction; v is computed token-major directly
(lhsT = hs tile, rhs = Wv slice) so no PE transpose or DRAM round-trip is
needed. Softmax has no max subtraction: P = exp(scale*S) * em where em is a
{0,1} (shifted-exp for general masks) factor; for exact-causal masks the 16
diagonal tiles per batch use only 4 distinct [128,512] patterns kept in SBUF.
Row sums via a PE ones-matmul; normalization is folded into the PSUM
eviction of the attention output. Output-projection blocks are interleaved
into the attention loop at lag-1 so the PE stays dense (HAM stays at 8/8).
"""

import os
import sys
from contextlib import ExitStack

import numpy as np

for _p in ("/opt/trn_rl_repo",):
    if _p not in sys.path:
        sys.path.insert(0, _p)

import ml_dtypes  # noqa: E402

import concourse.bass as bass  # noqa: E402
import concourse.tile as tile  # noqa: E402
from concourse import bacc, mybir  # noqa: E402

B, S, D, H, HD = 2, 2048, 2048, 16, 128
T = B * S                    # 4096 tokens total
NCORES = 8
HPC = H // NCORES            # 2 heads per core
JC = HPC * HD                # 256 per-core feature columns
P = 128
TB = 512                     # token block for projections
NTB = T // TB                # 8
KT = D // P                  # 16 contraction tiles of 128
TQB = 512                    # tq block in attention
NTQB = S // TQB              # 4 per batch
NTK = S // P                 # 16 tk tiles per batch
TTB = TB // P                # 4 token tiles of 128 per projection block
SCALE = 1.0 / float(np.sqrt(HD))
ROPE_THETA = 10000.0

F32 = mybir.dt.float32
BF16 = mybir.dt.bfloat16
F16 = mybir.dt.float16

# tile classes
CLS_SKIP, CLS_ZERO, CLS_MIXED = 0, 1, 2

_prog_cache: dict[bytes, tuple] = {}


def _build_program(cls: np.ndarray, causal: bool):
    """Build the SPMD Bass program. cls: [B, NTK, NTQB] int8 tile classes
    (identical for every core — the mask does not depend on the head).
    causal: the mixed tiles match the 4 canonical diagonal patterns."""
    has_mixed = bool((cls == CLS_MIXED).any())

    nc = bacc.Bacc(
        "TRN2",
        target_bir_lowering=False,
        debug=False,
        enable_asserts=True,
        num_devices=NCORES,
    )

    hs_d = nc.dram_tensor("hs", [NTB, P, KT * TB], BF16, kind="ExternalInput").ap()
    wq_d = nc.dram_tensor("wq", [P, KT * JC], BF16, kind="ExternalInput").ap()
    wk_d = nc.dram_tensor("wk", [P, KT * JC], BF16, kind="ExternalInput").ap()
    wv_d = nc.dram_tensor("wv", [P, KT * JC], BF16, kind="ExternalInput").ap()
    wo_d = nc.dram_tensor("wo", [P, HPC * D], BF16, kind="ExternalInput").ap()
    cos_d = nc.dram_tensor("cosT", [HD, T], BF16, kind="ExternalInput").ap()
    sin_d = nc.dram_tensor("sinT", [HD, T], BF16, kind="ExternalInput").ap()
    em_d = None
    if has_mixed:
        if causal:
            em_d = nc.dram_tensor("emT", [P, 4 * TQB], BF16, kind="ExternalInput").ap()
        else:
            em_d = nc.dram_tensor("emT", [B, S, S], BF16, kind="ExternalInput").ap()
    out_d = nc.dram_tensor("out", [T, D], BF16, kind="ExternalOutput").ap()

    with tile.TileContext(nc) as tc, ExitStack() as ctx:
        # ---------- long-lived tiles ----------
        persist = ctx.enter_context(tc.tile_pool(name="persist", bufs=1))
        qT = persist.tile([P, HPC * T], BF16)     # (j2, t) feature-major q
        kT = persist.tile([P, HPC * T], BF16)
        aT = persist.tile([P, HPC * T], BF16)     # attn out^T (j2, t)
        v_sb = persist.tile([P, B * NTK * JC], BF16)  # token-major v
        ones_t = persist.tile([P, 4], F16)
        nc.any.memset(ones_t[:], 1.0)

        qT_v = qT[:].rearrange("p (j2 t) -> p j2 t", j2=HPC)
        kT_v = kT[:].rearrange("p (j2 t) -> p j2 t", j2=HPC)
        aT_v = aT[:].rearrange("p (j2 t) -> p j2 t", j2=HPC)

        wpool = ctx.enter_context(tc.tile_pool(name="wpool", bufs=1))
        wq_s = wpool.tile([P, KT * JC], BF16)
        wk_s = wpool.tile([P, KT * JC], BF16)
        wv_s = wpool.tile([P, KT * JC], BF16)
        wo_s = wpool.tile([P, HPC * D], BF16)
        cos_s = wpool.tile([HD, T], BF16)
        sin_s = wpool.tile([HD, T], BF16)
        em4_s = None
        if has_mixed and causal:
            em4_s = wpool.tile([P, 4 * TQB], BF16)

        wq_v, wk_v, wv_v = [
            w[:].rearrange("p (kt j) -> p kt j", j=JC)
            for w in (wq_s, wk_s, wv_s)
        ]
        wo_sv = wo_s[:].rearrange("p (j2 n) -> p j2 n", j2=HPC)

        # ---------- shared work pools (both phases interleave) --------------
        hstp = ctx.enter_context(tc.tile_pool(name="hstp", bufs=2))
        rstg = ctx.enter_context(tc.tile_pool(name="rstg", bufs=3))
        ptp = ctx.enter_context(tc.tile_pool(name="ptp", bufs=6))
        emp = ctx.enter_context(tc.tile_pool(name="emp", bufs=2))
        ptap = ctx.enter_context(tc.tile_pool(name="ptap", bufs=2))
        ssbp = ctx.enter_context(tc.tile_pool(name="ssb", bufs=2))
        rcp = ctx.enter_context(tc.tile_pool(name="rcp", bufs=2))
        rbp = ctx.enter_context(tc.tile_pool(name="rbp", bufs=2))
        ogp = ctx.enter_context(tc.tile_pool(name="ogp", bufs=2))
        # PSUM: 8 banks total. "big" = q/k projection groups + attention
        # score tiles; "vop" = v projection groups + output projection.
        bigp = ctx.enter_context(tc.tile_pool(name="bigp", bufs=2,
                                              space="PSUM"))
        vopp = ctx.enter_context(tc.tile_pool(name="vopp", bufs=3,
                                              space="PSUM"))
        ops = ctx.enter_context(tc.tile_pool(name="opsum", bufs=1,
                                             space="PSUM"))
        sups = ctx.enter_context(tc.tile_pool(name="supsum", bufs=1,
                                              space="PSUM"))

        pend = {}

        def proj_block(tb, hst0=None):
            # q/k (feature-major + rope) and v (token-major) for 512 tokens
            if hst0 is not None:
                hst = hst0
            else:
                hst = hstp.tile([P, KT * TB], BF16, tag="hst")
                nc.sync.dma_start(hst[:], hs_d[tb])
            hv = hst[:].rearrange("p (kt t) -> p kt t", t=TB)
            tsl = slice(tb * TB, (tb + 1) * TB)
            for pi, w_v in ((0, wq_v), (1, wk_v)):
                for j2 in range(HPC):
                    ps = bigp.tile([P, TB], F32, tag="big")
                    for kt in range(KT):
                        nc.tensor.matmul(
                            ps[:],
                            lhsT=w_v[:, kt, j2 * P:(j2 + 1) * P],
                            rhs=hv[:, kt, :],
                            start=(kt == 0),
                            stop=(kt == KT - 1),
                        )
                    # rope folded into the PSUM eviction; sin_s is
                    # pre-signed (lower half negative) so one add works:
                    # out = x*cos + cross(x)*sin_signed
                    dst = (qT_v if pi == 0 else kT_v)[:, j2, tsl]
                    t1 = rstg.tile([P, TB], BF16, tag="ropeA")
                    t2 = rstg.tile([P, TB], BF16, tag="ropeB")
                    nc.vector.tensor_mul(t1[:], ps[:], cos_s[:, tsl])
                    nc.vector.tensor_mul(
                        t2[0:64, :], ps[64:128, :], sin_s[0:64, tsl])
                    nc.vector.tensor_mul(
                        t2[64:128, :], ps[0:64, :], sin_s[64:128, tsl])
                    nc.vector.tensor_add(dst, t1[:], t2[:])
                    pump()
            # v token-major: lhsT = hs tile, rhs = Wv slice
            for tt in range(TTB):
                vp = vopp.tile([P, JC], F32, tag="vop", name="vp")
                for kt in range(KT):
                    nc.tensor.matmul(
                        vp[:],
                        lhsT=hv[:, kt, tt * P:(tt + 1) * P],
                        rhs=wv_v[:, kt, :],
                        start=(kt == 0),
                        stop=(kt == KT - 1),
                    )
                g = tb * TTB + tt              # global token tile = b*NTK+tk
                nc.scalar.copy(v_sb[:, g * JC:(g + 1) * JC], vp[:])
                pump()

        def attn_block(b, tqb):
            tq0 = b * S + tqb * TQB
            qsl = slice(tq0, tq0 + TQB)
            live = [tk for tk in range(NTK) if cls[b, tk, tqb] != CLS_SKIP]
            o_ps = [ops.tile([P, TQB], F32, tag=f"o{h}", name=f"o_ps{h}")
                    for h in range(HPC)]
            # fp16 running sum of masked pt tiles (DVE); the softmax
            # denominators then need only ONE ones-matmul per (block, head)
            # instead of one per key tile.
            pta = [ptap.tile([P, TQB], F16, tag=f"pa{h}", name=f"pta{h}")
                   for h in range(HPC)]
            for i, tk in enumerate(live):
                mixed = cls[b, tk, tqb] == CLS_MIXED
                em_ap = None
                c0 = 0
                if mixed:
                    if causal:
                        r = tk - 4 * tqb
                        # the first 128*r query columns of a diagonal tile
                        # are fully masked: skip them in every engine
                        c0 = P * r
                        em_ap = em4_s[:, r * TQB + c0:(r + 1) * TQB]
                    else:
                        em = emp.tile([P, TQB], BF16, tag="em")
                        nc.sync.dma_start(
                            em[:],
                            em_d[b, tk * P:(tk + 1) * P,
                                 tqb * TQB:(tqb + 1) * TQB],
                        )
                        em_ap = em[:]
                W = TQB - c0
                pts = []
                for h in range(HPC):
                    st = bigp.tile([P, TQB], F32, tag="big", name="st")
                    nc.tensor.matmul(
                        st[:, 0:W],
                        lhsT=kT_v[:, h, b * S + tk * P:
                                  b * S + (tk + 1) * P],
                        rhs=qT_v[:, h, tq0 + c0:tq0 + TQB],
                        start=True, stop=True,
                    )
                    pt = ptp.tile([P, TQB], BF16, tag="pt")
                    nc.scalar.activation(
                        pt[:, c0:TQB], st[:, 0:W],
                        mybir.ActivationFunctionType.Exp,
                        scale=SCALE,
                    )
                    if em_ap is not None:
                        nc.vector.tensor_mul(pt[:, c0:TQB],
                                             pt[:, c0:TQB], em_ap)
                    if i == 0:
                        nc.vector.tensor_copy(pta[h][:], pt[:])
                    else:
                        nc.vector.tensor_add(pta[h][:, c0:TQB],
                                             pta[h][:, c0:TQB],
                                             pt[:, c0:TQB])
                    pts.append(pt)
                for h in range(HPC):
                    nc.tensor.matmul(
                        o_ps[h][:, c0:TQB],
                        lhsT=v_sb[:, (b * NTK + tk) * JC + h * P:
                                  (b * NTK + tk) * JC + (h + 1) * P],
                        rhs=pts[h][:, c0:TQB],
                        start=(i == 0), stop=(i == len(live) - 1),
                    )
                # spread outproj fill evenly across the block instead of
                # draining the queue in the first iterations
                if i % max(1, len(live) // 4) == 0:
                    pump()
            # denominators: one 512-col ones-matmul per head; both heads'
            # [1,TQB] rows share one PSUM bank on disjoint partition rows
            # (0 and 64), so each is its own accumulation group.
            s_ps2 = sups.tile([P, TQB], F32, tag="s2", name="s_ps2")
            for h in range(HPC):
                nc.tensor.matmul(
                    s_ps2[64 * h:64 * h + 1, :],
                    lhsT=ones_t[:, 0:1],
                    rhs=pta[h][:],
                    start=True, stop=True,
                )
            # split the four end-of-block evictions across ACT and DVE so
            # the next block's first exp isn't queued behind all of them
            # split the four end-of-block evictions across ACT and DVE so
            # the next block's first exp isn't queued behind all of them
            ssb = []
            nc.scalar.copy(aT_v[:, 0, qsl], o_ps[0][:])
            nc.vector.tensor_copy(aT_v[:, 1, qsl], o_ps[1][:])
            for h in range(HPC):
                s_sb = ssbp.tile([1, TQB], F32, tag=f"ssb{h}")
                if h == 0:
                    nc.vector.tensor_copy(s_sb[:], s_ps2[0:1, :])
                else:
                    nc.scalar.copy(s_sb[:], s_ps2[64:65, :])
                ssb.append(s_sb)
            pend[(b, tqb)] = ssb

        def chain_block(b, tqb):
            tq0 = b * S + tqb * TQB
            qsl = slice(tq0, tq0 + TQB)
            ssb = pend.pop((b, tqb))
            for h in range(HPC):
                rc = rcp.tile([1, TQB], F32, tag=f"rc{h}")
                nc.vector.reciprocal_approx_fast(rc[:], ssb[h][:])
                rb = rbp.tile([P, TQB], F32, tag=f"rb{h}")
                nc.gpsimd.partition_broadcast(rb[:], rc[:])
                nc.vector.tensor_mul(aT_v[:, h, qsl],
                                     aT_v[:, h, qsl], rb[:])

        outq = []

        def out_unit(b, tqb, tt):
            # one token tile of output projection; aT is already normalized,
            # so the two heads accumulate in one PSUM group and evictions
            # alternate ACT/DVE so neither FIFO gates the PE.
            tb32 = (b * S + tqb * TQB) // P + tt
            og = ogp.tile([P, D], BF16, tag="og")
            for nb in range(D // 512):
                op = vopp.tile([P, 512], F32, tag="vop", name="op")
                for h in range(HPC):
                    nc.tensor.matmul(
                        op[:],
                        lhsT=aT_v[:, h, tb32 * P:(tb32 + 1) * P],
                        rhs=wo_sv[:, h, nb * 512:(nb + 1) * 512],
                        start=(h == 0), stop=(h == HPC - 1),
                    )
                osl = og[:, nb * 512:(nb + 1) * 512]
                if nb % 2 == 0:
                    nc.vector.tensor_copy(osl, op[:])
                else:
                    nc.scalar.copy(osl, op[:])
            nc.sync.dma_start(
                out_d[tb32 * P:(tb32 + 1) * P, :], og[:])

        def out_block(b, tqb):
            # queue the 4 token-tile units; they are traced ("pumped") one at
            # a time between attention tiles / projection groups so bubbles
            # in those pipelines get filled with independent outproj matmuls
            for tt in range(TQB // P):
                outq.append((b, tqb, tt))

        def pump():
            if outq:
                out_unit(*outq.pop(0))

        # ---------- schedule -----------------------------------------------
        # proj(b0) -> attn(b0) (outproj lag-1) -> proj(b1) -> attn(b1).
        # attention(b0)'s softmax/eviction chains drain on ACT/DVE/GPSIMD
        # while projection(b1) keeps the PE dense, and vice versa.
        # startup DMAs chunked and ordered by first use so the PE can start
        # streaming the first projection group at ~1us instead of waiting for
        # whole-tensor transfers: interleave wq/hs0 per-kt chunks, then the
        # rope tables for block 0's evictions, then wk, wv, and the rest.
        hst0 = hstp.tile([P, KT * TB], BF16, tag="hst")
        nc.sync.dma_start(wq_s[:], wq_d)
        nc.sync.dma_start(hst0[:], hs_d[0])
        nc.sync.dma_start(cos_s[:], cos_d)
        nc.sync.dma_start(sin_s[:], sin_d)
        nc.sync.dma_start(wk_s[:], wk_d)
        nc.sync.dma_start(wv_s[:], wv_d)
        nc.sync.dma_start(wo_s[:], wo_d)
        if em4_s is not None:
            nc.sync.dma_start(em4_s[:], em_d)
        proj_block(0, hst0=hst0)
        for tb in range(1, NTB // 2):
            proj_block(tb)
        blocks0 = [(0, tqb) for tqb in range(NTQB)]
        blocks1 = [(1, tqb) for tqb in range(NTQB)]
        for k, blk in enumerate(blocks0):
            attn_block(*blk)
            if k >= 1:
                chain_block(*blocks0[k - 1])
                out_block(*blocks0[k - 1])
        proj_block(NTB // 2)
        chain_block(*blocks0[-1])
        out_block(*blocks0[-1])
        for tb in range(NTB // 2 + 1, NTB):
            proj_block(tb)
        while outq:
            pump()
        for k, blk in enumerate(blocks1):
            attn_block(*blk)
            if k >= 1:
                chain_block(*blocks1[k - 1])
                out_block(*blocks1[k - 1])
        chain_block(*blocks1[-1])
        out_block(*blocks1[-1])
        while outq:
            pump()

    nc.compile()
    return nc


def _host_prep(hidden_states, attention_mask, position_ids):
    hs = np.asarray(hidden_states, dtype=np.float32).reshape(T, D)
    # hs_d[tb, p, kt*TB + t] = hs[tb*TB + t, kt*P + p]
    hs_t = np.ascontiguousarray(
        hs.reshape(NTB, TB, KT, P).transpose(0, 3, 2, 1)
        .reshape(NTB, P, KT * TB).astype(ml_dtypes.bfloat16))

    # rope tables gathered by position_ids, feature-major, sign baked into sin
    inv_freq = 1.0 / (ROPE_THETA ** (np.arange(0, HD, 2, dtype=np.float32) / HD))
    pos = np.asarray(position_ids).astype(np.int64)
    maxpos = int(pos.max()) + 1
    t_ar = np.arange(maxpos, dtype=np.float32)
    freqs = np.outer(t_ar, inv_freq)                        # [maxpos, 64]
    emb = np.concatenate([freqs, freqs], axis=-1)           # [maxpos, 128]
    cos_tab = np.cos(emb).astype(np.float32)
    sin_tab = np.sin(emb).astype(np.float32)
    sin_tab[:, :HD // 2] *= -1.0                            # pre-signed
    cos_g = cos_tab[pos]                                    # [B, S, HD]
    sin_g = sin_tab[pos]
    cosT = np.concatenate([cos_g[b].T for b in range(B)], axis=1)  # [HD, T]
    sinT = np.concatenate([sin_g[b].T for b in range(B)], axis=1)
    cosT = np.ascontiguousarray(cosT.astype(ml_dtypes.bfloat16))
    sinT = np.ascontiguousarray(sinT.astype(ml_dtypes.bfloat16))

    # shifted-exp mask, transposed per batch, plus tile classification
    m = np.asarray(attention_mask, dtype=np.float32)[:, 0]  # [B, S(tq), S(tk)]
    rowmax = m.max(axis=-1, keepdims=True)
    em = np.exp(m - rowmax)                                 # [B, tq, tk] in [0,1]
    emT = np.ascontiguousarray(em.transpose(0, 2, 1))       # [B, tk, tq]
    emr = emT.reshape(B, NTK, P, NTQB, TQB)
    tmax = emr.max(axis=(2, 4))                             # [B, NTK, NTQB]
    tmin = emr.min(axis=(2, 4))
    cls = np.full((B, NTK, NTQB), CLS_MIXED, dtype=np.int8)
    cls[tmax == 0.0] = CLS_SKIP
    cls[(tmin == 1.0) & (tmax == 1.0)] = CLS_ZERO
    # guard: a fully-skipped tq column would leave PSUM unwritten
    for b in range(B):
        for tqb in range(NTQB):
            if (cls[b, :, tqb] == CLS_SKIP).all():
                cls[b, 0, tqb] = CLS_MIXED

    # causal fast path: every mixed tile sits on the diagonal (tk = 4*tqb+r,
    # r in 0..3) and matches the canonical pattern 1[tql >= tkl + 128*r]
    tkl = np.arange(P)[:, None]
    tql = np.arange(TQB)[None, :]
    pat = np.stack([(tql >= tkl + P * r).astype(np.float32)
                    for r in range(4)])                     # [4, P, TQB]
    causal = True
    for b in range(B):
        for tk in range(NTK):
            for tqb in range(NTQB):
                if cls[b, tk, tqb] != CLS_MIXED:
                    continue
                r = tk - 4 * tqb
                tile_ = emr[b, tk, :, tqb, :]
                if not (0 <= r < 4) or not np.array_equal(tile_, pat[r]):
                    causal = False
    em_out = (np.ascontiguousarray(
                  pat.transpose(1, 0, 2).reshape(P, 4 * TQB)
                  .astype(ml_dtypes.bfloat16)) if causal
              else np.ascontiguousarray(emT.astype(ml_dtypes.bfloat16)))
    return hs_t, cosT, sinT, em_out, cls, causal


def kernel(hidden_states, attention_mask, position_ids, Wq, Wk, Wv, Wo):
    hs_t, cosT, sinT, em_out, cls, causal = _host_prep(
        hidden_states, attention_mask, position_ids)

    key = cls.tobytes() + bytes([causal])
    if key not in _prog_cache:
        _prog_cache[key] = _build_program(cls, causal)
    nc = _prog_cache[key]
    has_mixed = bool((cls == CLS_MIXED).any())

    Wq = np.asarray(Wq, dtype=np.float32)
    Wk = np.asarray(Wk, dtype=np.float32)
    Wv = np.asarray(Wv, dtype=np.float32)
    Wo = np.asarray(Wo, dtype=np.float32)

    in_maps = []
    for c in range(NCORES):
        jsl = slice(c * JC, (c + 1) * JC)

        def sb_w(w):  # [D, JC] -> SBUF layout [128, KT*JC]
            return np.ascontiguousarray(
                w.reshape(KT, P, JC).transpose(1, 0, 2).reshape(P, KT * JC)
                .astype(ml_dtypes.bfloat16))

        m = {
            "hs": hs_t,
            "wq": sb_w(Wq[:, jsl]),
            "wk": sb_w(Wk[:, jsl]),
            "wv": sb_w(Wv[:, jsl]),
            "wo": np.ascontiguousarray(
                Wo[jsl, :].reshape(HPC, P, D).transpose(1, 0, 2)
                .reshape(P, HPC * D).astype(ml_dtypes.bfloat16)),
            "cosT": cosT,
            "sinT": sinT,
        }
        if has_mixed:
            m["emT"] = em_out
        in_maps.append(m)

    if os.environ.get("KERNEL_SIM"):
        from concourse.bass_interp import CoreSim
        outs = []
        for c in range(int(os.environ.get("KERNEL_SIM_CORES", "1"))):
            sim = CoreSim(nc, require_finite=False, require_nnan=True)
            for k, v in in_maps[c].items():
                sim.tensor(k)[:] = v
            sim.simulate(check_with_hw=False)
            outs.append(np.array(sim.tensor("out")).astype(np.float32))
        total = np.sum(np.stack(outs, 0), axis=0)
        return total.reshape(B, S, D).astype(np.float32)

    from concourse.bass_utils import run_bass_kernel_spmd
    trace = bool(os.environ.get("KERNEL_TRACE"))
    res = run_bass_kernel_spmd(
        nc, in_maps, core_ids=list(range(NCORES)), trace=trace)
    if trace and res.exec_time_ns is not None:
        print(f"HW exec time: {res.exec_time_ns} ns")
        kernel.last_exec_time_ns = res.exec_time_ns
        kernel.last_trace = res.instructions_and_trace
    partials = np.stack([r["out"].astype(np.float32) for r in res.results],
                        axis=0)
    total = partials.sum(axis=0)
    return total.reshape(B, S, D).astype(np.float32)

